# revision 9
# baseline (speedup 1.0000x reference)
"""Trainium2 Bass kernel for a 2-layer GCN (GCNConv+ReLU+BN x2, mean-pool).

Strategy (8 NeuronCores, SPMD):
- Dest-node sharding: each core owns NB=ceil(N/8/128) blocks of 128 nodes.
- Message passing out[c] = sum_e w_e * h[src_e] (w_e = dinv[r]*dinv[c],
  self-loops included as edges) is computed per dest block as a chain of
  one-hot matmuls on the TensorEngine: psum += E_t^T @ msg_t, where msg_t is
  128 source rows fetched with dma_gather (fp16 rows padded to 512B) and
  E_t[e, d] = (d_e == d) * w_e built on the VectorEngine via iota/is_equal.
- The weight multiply commutes with aggregation, so x@W never happens
  up front: per block, agg is transposed on the PE and multiplied by W
  (fp16, f32 accum), bias added via a ones-row matmul, then ReLU (ScalarE)
  and the folded BatchNorm affine (VectorE).
- Layer-1 results are written per-core and AllGathered (on-chip collective)
  into a shared buffer that layer 2 gathers from.
- Mean-pool: per block a one-hot P matmul reduces 128 nodes into <=128
  graph partials; the host sums overlapping block partials and divides by
  graph counts (the cross-core unshard step).
"""
import os
import numpy as np
from contextlib import ExitStack

import concourse.bacc as bacc
import concourse.bass as bass
import concourse.mybir as mybir
import concourse.tile as tile
from concourse.library_config import mlp
from concourse.bass_utils import run_bass_kernel_spmd

dt = mybir.dt
NCORES = 8
PB = 128          # nodes per dest block
EW = 256          # fp16 elements per padded row (512 bytes)
EPS = 1e-5
G_OUT = 2048      # number of graphs in the output


# ---------------------------------------------------------------- host prep
def preprocess(x, edge_index, batch, force_T=None):
    N, D = x.shape
    G = G_OUT
    NB = -(-N // (NCORES * PB))          # blocks per core
    S = NB * PB
    TOT = NCORES * S
    L = TOT // 2
    assert L <= 32768 and TOT - L <= 32768

    r = np.asarray(edge_index[0], dtype=np.int64)
    c = np.asarray(edge_index[1], dtype=np.int64)
    loops = np.arange(N, dtype=np.int64)
    r = np.concatenate([r, loops])
    c = np.concatenate([c, loops])
    deg = np.bincount(c, minlength=N).astype(np.float64)
    dinv = np.where(deg > 0, 1.0 / np.sqrt(deg), 0.0)
    w = (dinv[r] * dinv[c]).astype(np.float32)

    half = (r >= L).astype(np.int64)
    src = np.where(half == 0, r, r - L).astype(np.int64)
    blk = c // PB
    d = (c % PB).astype(np.int64)

    ngroups = NCORES * NB * 2
    key = blk * 2 + half
    order = np.argsort(key, kind="stable")
    src, d, w, key = src[order], d[order], w[order], key[order]
    counts = np.bincount(key, minlength=ngroups)
    T = max(1, int(-(-counts.max() // PB)))
    if force_T is not None:
        assert force_T >= T
        T = force_T
    gsz = T * PB

    src_p = np.zeros((ngroups, gsz), dtype=np.int16)
    d_p = np.zeros((ngroups, gsz), dtype=np.float32)
    w_p = np.zeros((ngroups, gsz), dtype=np.float32)
    starts = np.concatenate([[0], np.cumsum(counts)])
    for g in range(ngroups):
        n = counts[g]
        src_p[g, :n] = src[starts[g]:starts[g] + n]
        d_p[g, :n] = d[starts[g]:starts[g] + n]
        w_p[g, :n] = w[starts[g]:starts[g] + n]

    # per-core device arrays
    G2 = NB * 2
    idx_arr = np.zeros((NCORES, 128, G2 * 8 * T), dtype=np.int16)
    d_arr = np.zeros((NCORES, 128, G2 * T), dtype=np.float32)
    w_arr = np.zeros((NCORES, 128, G2 * T), dtype=np.float32)
    for k in range(NCORES):
        for gl in range(G2):
            g = k * G2 + gl
            # dma_gather index layout: index j lives at [j % 16, j // 16]
            wrapped = src_p[g].reshape(8 * T, 16).T            # [16, 8T]
            idx_arr[k, :, gl * 8 * T:(gl + 1) * 8 * T] = np.tile(wrapped, (8, 1))
            d_arr[k, :, gl * T:(gl + 1) * T] = d_p[g].reshape(T, PB).T
            w_arr[k, :, gl * T:(gl + 1) * T] = w_p[g].reshape(T, PB).T

    # pooling: per (core, block) graph base + local graph ids
    batch_pad = np.full(TOT, -1, dtype=np.int64)
    batch_pad[:N] = batch
    blocks = batch_pad.reshape(NCORES * NB, PB)
    valid = blocks >= 0
    base = np.where(valid.any(axis=1),
                    np.where(valid, blocks, np.iinfo(np.int64).max).min(axis=1),
                    0)
    bloc = np.where(valid, blocks - base[:, None], -1).astype(np.float32)
    bloc_arr = bloc.reshape(NCORES, NB, PB).transpose(0, 2, 1).copy()  # [k,128,NB]

    cnts = np.bincount(batch, minlength=G).astype(np.float32)
    return dict(N=N, D=D, G=G, NB=NB, S=S, TOT=TOT, L=L, T=T,
                idx_arr=idx_arr, d_arr=d_arr, w_arr=w_arr,
                bloc_arr=bloc_arr, base=base, cnts=cnts)


def fold_bn(g, beta, rm, rv):
    gp = (g / np.sqrt(rv + EPS)).astype(np.float32)
    bp = (beta - rm * gp).astype(np.float32)
    return gp, bp


# ---------------------------------------------------------------- bass build
def build_nc(NB, T, D, TOT, S, L):
    f16, f32, i16 = dt.float16, dt.float32, dt.int16
    G2 = NB * 2
    nc = bacc.Bacc("TRN2", target_bir_lowering=False, debug=False,
                   num_devices=NCORES)

    xbuf = nc.dram_tensor("xbuf", [TOT, EW], f16, kind="ExternalInput")
    idxt = nc.dram_tensor("idxt", [128, G2 * 8 * T], i16, kind="ExternalInput")
    dcol = nc.dram_tensor("dcol", [128, G2 * T], f32, kind="ExternalInput")
    wcol = nc.dram_tensor("wcol", [128, G2 * T], f32, kind="ExternalInput")
    bcol = nc.dram_tensor("bcol", [128, NB], f32, kind="ExternalInput")
    iot = nc.dram_tensor("iot", [128, 128], f16, kind="ExternalInput")
    idn = nc.dram_tensor("idn", [128, 128], f32, kind="ExternalInput")
    onesr = nc.dram_tensor("onesr", [1, 128], f16, kind="ExternalInput")
    whi = nc.dram_tensor("whi", [2, 128, D], f16, kind="ExternalInput")
    wlo = nc.dram_tensor("wlo", [2, D - 128, D], f16, kind="ExternalInput")
    brow = nc.dram_tensor("brow", [2, 1, D], f16, kind="ExternalInput")
    gam = nc.dram_tensor("gam", [2, 128, D], f32, kind="ExternalInput")
    bet = nc.dram_tensor("bet", [2, 128, D], f32, kind="ExternalInput")
    outp = nc.dram_tensor("outp", [NB * PB, D], f32, kind="ExternalOutput")
    h1sl = nc.dram_tensor("h1sl", [S, EW], f16, kind="Internal")
    h1f = nc.dram_tensor("h1f", [TOT, EW], f16, kind="Internal",
                         addr_space="Shared")

    DLO = D - 128
    with tile.TileContext(nc) as tc, ExitStack() as ctx:
        cp = ctx.enter_context(tc.tile_pool(name="consts", bufs=1))
        gp_ = ctx.enter_context(tc.tile_pool(name="gath", bufs=4))
        ep = ctx.enter_context(tc.tile_pool(name="onehot", bufs=4))
        sp = ctx.enter_context(tc.tile_pool(name="stage", bufs=3))
        pagg = ctx.enter_context(tc.tile_pool(name="pagg", bufs=2, space="PSUM"))
        ptr = ctx.enter_context(tc.tile_pool(name="ptr", bufs=2, space="PSUM"))
        ph = ctx.enter_context(tc.tile_pool(name="ph", bufs=2, space="PSUM"))
        ppool = ctx.enter_context(tc.tile_pool(name="ppool", bufs=2, space="PSUM"))

        def load_const(name, dram, shape, dtype):
            t = cp.tile(shape, dtype, name=name)
            nc.sync.dma_start(t[:], dram)
            return t

        idxS = load_const("idxS", idxt[:, :], [128, G2 * 8 * T], i16)
        dS = load_const("dS", dcol[:, :], [128, G2 * T], f32)
        wS = load_const("wS", wcol[:, :], [128, G2 * T], f32)
        bS = load_const("bS", bcol[:, :], [128, NB], f32)
        iotS = load_const("iotS", iot[:, :], [128, 128], f16)
        idnS = load_const("idnS", idn[:, :], [128, 128], f32)
        onesS = load_const("onesS", onesr[:, :], [1, 128], f16)
        whiS = [load_const(f"whiS{l}", whi[l, :, :], [128, D], f16) for l in range(2)]
        wloS = [load_const(f"wloS{l}", wlo[l, :, :], [DLO, D], f16) for l in range(2)]
        browS = [load_const(f"browS{l}", brow[l, :, :], [1, D], f16) for l in range(2)]
        gamS = [load_const(f"gamS{l}", gam[l, :, :], [128, D], f32) for l in range(2)]
        betS = [load_const(f"betS{l}", bet[l, :, :], [128, D], f32) for l in range(2)]

        nc.gpsimd.load_library(mlp)

        for layer in range(2):
            src = xbuf if layer == 0 else h1f
            for b in range(NB):
                agg = pagg.tile([128, D], f32, name=f"agg_{layer}_{b}", tag="agg")
                for h in range(2):
                    gl = b * 2 + h
                    gt = gp_.tile([128, T, EW], f16, name=f"gt_{layer}_{gl}",
                                  tag="gt")
                    in_ap = src[0:TOT, :] if h == 0 else src[L:TOT, :]
                    nc.gpsimd.dma_gather(
                        gt[:], in_ap, idxS[:, gl * 8 * T:(gl + 1) * 8 * T],
                        T * PB, T * PB, EW, single_packet=False)
                    for t in range(T):
                        cc = gl * T + t
                        E = ep.tile([128, 128], f16, name=f"E_{layer}_{cc}",
                                    tag="E")
                        nc.vector.tensor_scalar(
                            E[:], iotS[:], dS[:, cc:cc + 1], wS[:, cc:cc + 1],
                            op0=mybir.AluOpType.is_equal,
                            op1=mybir.AluOpType.mult)
                        nc.tensor.matmul(
                            agg[:], E[:], gt[:, t, 0:D],
                            start=(h == 0 and t == 0),
                            stop=(h == 1 and t == T - 1))
                # epilogue: transpose agg, @W, +b, relu, BN affine
                aggS = sp.tile([128, D], f32, name=f"aggS_{layer}_{b}", tag="aggS")
                nc.vector.tensor_copy(aggS[:], agg[:])
                psT = ptr.tile([128, 256], f32, name=f"psT_{layer}_{b}", tag="psT")
                nc.tensor.transpose(psT[:, 0:128], aggS[:, 0:128], idnS[:])
                nc.tensor.transpose(psT[0:DLO, 128:256], aggS[:, 128:D], idnS[:])
                t1 = sp.tile([128, 128], f16, name=f"t1_{layer}_{b}", tag="t1")
                nc.vector.tensor_copy(t1[:], psT[:, 0:128])
                t2 = sp.tile([DLO, 128], f16, name=f"t2_{layer}_{b}", tag="t2")
                nc.vector.tensor_copy(t2[:], psT[0:DLO, 128:256])
                zps = ph.tile([128, D], f32, name=f"zps_{layer}_{b}", tag="zps")
                nc.tensor.matmul(zps[:], t1[:], whiS[layer][:],
                                 start=True, stop=False)
                nc.tensor.matmul(zps[:], t2[:], wloS[layer][:],
                                 start=False, stop=False)
                nc.tensor.matmul(zps[:], onesS[:], browS[layer][:],
                                 start=False, stop=True)
                rl = sp.tile([128, D], f32, name=f"rl_{layer}_{b}", tag="rl")
                nc.scalar.activation(rl[:], zps[:],
                                     mybir.ActivationFunctionType.Relu)
                m1 = sp.tile([128, D], f32, name=f"m1_{layer}_{b}", tag="m1")
                nc.vector.tensor_mul(m1[:], rl[:], gamS[layer][:])
                hS = sp.tile([128, D], f16, name=f"hS_{layer}_{b}", tag="hS")
                nc.vector.tensor_add(hS[:], m1[:], betS[layer][:])
                if layer == 0:
                    nc.sync.dma_start(h1sl[b * PB:(b + 1) * PB, 0:D], hS[:])
                else:
                    P = ep.tile([128, 128], f16, name=f"P_{b}", tag="E")
                    nc.vector.tensor_scalar(
                        P[:], iotS[:], bS[:, b:b + 1], None,
                        op0=mybir.AluOpType.is_equal)
                    pps = ppool.tile([128, D], f32, name=f"pps_{b}", tag="pps")
                    nc.tensor.matmul(pps[:], P[:], hS[:], start=True, stop=True)
                    po = sp.tile([128, D], f32, name=f"po_{b}", tag="po")
                    nc.vector.tensor_copy(po[:], pps[:])
                    nc.sync.dma_start(outp[b * PB:(b + 1) * PB, :], po[:])
            if layer == 0:
                nc.gpsimd.collective_compute(
                    "AllGather", mybir.AluOpType.bypass,
                    replica_groups=[list(range(NCORES))],
                    ins=[h1sl[:, :].opt()], outs=[h1f[:, :].opt()])

    nc.compile()
    return nc


# ---------------------------------------------------------------- entry
_NC_CACHE = {}


def kernel(x, edge_index, batch, W1, b1, W2, b2,
           g1, beta1, rm1, rv1, g2, beta2, rm2, rv2):
    nc, in_maps, pp = prepare(x, edge_index, batch, W1, b1, W2, b2,
                              g1, beta1, rm1, rv1, g2, beta2, rm2, rv2)
    res = run_bass_kernel_spmd(nc, in_maps, core_ids=list(range(NCORES)))
    return combine(pp, [res.results[k]["outp"] for k in range(NCORES)])


def prepare(x, edge_index, batch, W1, b1, W2, b2,
            g1, beta1, rm1, rv1, g2, beta2, rm2, rv2):
    """Build (nc, in_maps, pp) without running — used by the benchmark."""
    x = np.asarray(x, dtype=np.float32)
    pp = preprocess(x, np.asarray(edge_index), np.asarray(batch))
    D = pp["D"]
    key = (pp["NB"], pp["T"], D, pp["TOT"], pp["S"], pp["L"])
    if key not in _NC_CACHE:
        _NC_CACHE[key] = build_nc(*key)
    nc = _NC_CACHE[key]

    xbuf = np.zeros((pp["TOT"], EW), dtype=np.float16)
    xbuf[:pp["N"], :D] = x.astype(np.float16)
    iot = np.broadcast_to(np.arange(128, dtype=np.float16), (128, 128)).copy()
    idn = np.eye(128, dtype=np.float32)
    onesr = np.ones((1, 128), dtype=np.float16)
    g1p, b1p = fold_bn(g1, beta1, rm1, rv1)
    g2p, b2p = fold_bn(g2, beta2, rm2, rv2)
    whi = np.stack([W1[:128], W2[:128]]).astype(np.float16)
    wlo = np.stack([W1[128:], W2[128:]]).astype(np.float16)
    brow = np.stack([b1[None, :], b2[None, :]]).astype(np.float16)
    gam = np.stack([np.broadcast_to(g1p, (128, D)),
                    np.broadcast_to(g2p, (128, D))]).astype(np.float32)
    bet = np.stack([np.broadcast_to(b1p, (128, D)),
                    np.broadcast_to(b2p, (128, D))]).astype(np.float32)
    in_maps = []
    for k in range(NCORES):
        in_maps.append({
            "xbuf": xbuf, "idxt": pp["idx_arr"][k], "dcol": pp["d_arr"][k],
            "wcol": pp["w_arr"][k], "bcol": pp["bloc_arr"][k],
            "iot": iot, "idn": idn, "onesr": onesr,
            "whi": whi, "wlo": wlo, "brow": brow, "gam": gam, "bet": bet,
        })
    return nc, in_maps, pp


def combine(pp, outs):
    sums = np.zeros((pp["G"] + PB, pp["D"]), dtype=np.float32)
    for k in range(NCORES):
        o = outs[k]
        for b in range(pp["NB"]):
            bb = pp["base"][k * pp["NB"] + b]
            sums[bb:bb + PB] += o[b * PB:(b + 1) * PB]
    return (sums[:pp["G"]]
            / np.maximum(pp["cnts"], 1.0)[:, None]).astype(np.float32)
